# revision 11
# baseline (speedup 1.0000x reference)
"""Single-head causal attention (B=8, T=2048, C=1024, H=64) on 8 TRN2 NeuronCores.

Sharding: data-parallel over batch -- core b computes batch element b. No
collectives. Per core, for x_b [T, C]:
    q = x_b @ Wq / sqrt(H); k = x_b @ Wk; v = x_b @ Wv
    out = softmax(causal(q @ k.T)) @ v

v4 design (v2 ~66.6 us, fp32r baseline ~102 us). On this toolchain the PE
serializes LDWEIGHTS with the preceding matmul (single weight buffer;
walrus' ldw-opt pass is disabled), so array tile-packing does NOT overlap
matmuls -- the v2 matmul stream is already near the serial floor. v4 keeps
v2's matmul structure and attacks startup latency, engine-queue pressure
and the tail instead:
  - All matmul operands bf16; PSUM accumulation fp32.
  - Host pre-shuffles x into xh[p, g, ci, t]. Per-group SBUF tiles; group
    0 arrives as two half-DMAs so the first projection starts ~6 us
    earlier. Weights + constants ride a different DMA ring (scalar's)
    than the bulk x transfers (sync's).
  - ACT does exp only; k-tile pairs share one [128, 2, 512] PSUM tile and
    full (non-diagonal) pairs get a single merged exp call, halving ACT
    instruction+semaphore pressure.
  - All PSUM->SBUF casts/copies/masks are explicit DVE ops.
  - Softmax denominator rides row 64 of the O matmul (ones column in V).
    Output is UNNORMALIZED [65, T] bf16 (64 rows O^T, row 64 = denom);
    the division happens on the host. No reciprocal / broadcast on device.
"""

from contextlib import ExitStack

import numpy as np
import ml_dtypes

import concourse.mybir as mybir
import concourse.tile as tile
from concourse import bacc
from concourse.bass_utils import run_bass_kernel_spmd
from concourse.masks import make_identity, make_upper_triangular

B, T, C, H = 8, 2048, 1024, 64
N_CORES = 8
GQ = 512          # q-group width (PSUM bank)
NG = T // GQ      # 4 q-groups
KT = 128          # k-tile size
CC = C // 128     # 8 contraction chunks
F32 = mybir.dt.float32
BF16 = mybir.dt.bfloat16
EXP = mybir.ActivationFunctionType.Exp
BF_NP = ml_dtypes.bfloat16


def _emit(ctx, tc):
    nc = tc.nc
    xh = nc.dram_tensor("xh", [128, NG, CC, GQ], BF16, kind="ExternalInput").ap()
    wqk = nc.dram_tensor("wqk", [128, CC, 2 * H], BF16, kind="ExternalInput").ap()
    wv = nc.dram_tensor("wv", [128, CC, H], BF16, kind="ExternalInput").ap()
    # rows 0..63: unnormalized O^T; row 64: softmax denominator
    outT = nc.dram_tensor("outT", [H + 1, T], BF16, kind="ExternalOutput").ap()

    const = ctx.enter_context(tc.tile_pool(name="const", bufs=1))
    persist = ctx.enter_context(tc.tile_pool(name="persist", bufs=1))
    pt_pool = ctx.enter_context(tc.tile_pool(name="pt", bufs=4))
    out_pool = ctx.enter_context(tc.tile_pool(name="outp", bufs=2))
    # PSUM (8 banks): qk 2 + v 1 + s-pairs 2x2 (transposes share) + o 1
    ps_qk = ctx.enter_context(tc.tile_pool(name="ps_qk", bufs=2, space="PSUM"))
    ps_v = ctx.enter_context(tc.tile_pool(name="ps_v", bufs=1, space="PSUM"))
    ps_s = ctx.enter_context(tc.tile_pool(name="ps_s", bufs=2, space="PSUM"))
    ps_o = ctx.enter_context(tc.tile_pool(name="ps_o", bufs=1, space="PSUM"))

    # weights + constants on the scalar engine's DMA ring so they don't
    # queue behind the 1 MB x transfers on sync's ring
    wqk_sb = const.tile([128, CC, 2 * H], BF16)
    nc.scalar.dma_start(out=wqk_sb[:], in_=wqk)
    wv_sb = const.tile([128, CC, H], BF16)
    nc.scalar.dma_start(out=wv_sb[:], in_=wv)
    # mask[p, f] = 1.0 where p <= f else 0 : keep k_local <= q_local.
    mask_f = const.tile([128, 128], F32)
    make_upper_triangular(nc, mask_f[:], val=1.0, diag=True)
    mask = const.tile([128, 128], BF16)
    nc.vector.tensor_copy(mask[:], mask_f[:])
    ident_f = const.tile([H, H], F32)
    make_identity(nc, ident_f[:])
    ident = const.tile([H, H], BF16)
    nc.vector.tensor_copy(ident[:], ident_f[:])

    # per-group x tiles; group 0 in two half-DMAs so ci=0..3 land early
    xsb = []
    for g in range(NG):
        xg = persist.tile([128, CC, GQ], BF16, tag=f"x{g}")
        if g == 0:
            nc.sync.dma_start(out=xg[:, 0 : CC // 2], in_=xh[:, g, 0 : CC // 2])
            nc.sync.dma_start(out=xg[:, CC // 2 : CC], in_=xh[:, g, CC // 2 : CC])
        else:
            nc.sync.dma_start(out=xg[:], in_=xh[:, g])
        xsb.append(xg)

    qt = persist.tile([H, T], BF16)             # Q^T (pre-scaled by 1/sqrt(H))
    kt = persist.tile([H, T], BF16)             # K^T
    vt = persist.tile([H, T], BF16)             # V^T
    vsb = persist.tile([128, T // KT, H + 1], BF16)  # V natural tiles + ones col
    nc.vector.memset(vsb[:, :, H : H + 1], 1.0)

    for g in range(NG):
        sl = slice(GQ * g, GQ * (g + 1))
        # ---- projections for t-span sl ----
        qk_ps = ps_qk.tile([128, GQ], F32)
        v_ps = ps_v.tile([H, GQ], F32)
        for ci in range(CC):
            nc.tensor.matmul(qk_ps[:], wqk_sb[:, ci, :], xsb[g][:, ci],
                             start=(ci == 0), stop=(ci == CC - 1))
            nc.tensor.matmul(v_ps[:], wv_sb[:, ci, :], xsb[g][:, ci],
                             start=(ci == 0), stop=(ci == CC - 1))
        nc.vector.tensor_copy(qt[:, sl], qk_ps[0:H, :])
        nc.vector.tensor_copy(kt[:, sl], qk_ps[H:128, :])
        nc.vector.tensor_copy(vt[:, sl], v_ps[:, :])
        # ---- V^T -> natural V tiles (PE transpose, pairs share an s-slot) ----
        for jj in range(0, 4, 2):
            j0, j1 = 4 * g + jj, 4 * g + jj + 1
            # [128, 2, 1024] bf16 = 2KB per half: each transpose's start=True
            # zeroes a whole PSUM bank, so each half gets its own bank
            tr_ps = ps_s.tile([128, 2, 1024], BF16, tag="s")
            nc.tensor.transpose(tr_ps[:, 0, 0:H], vt[:, KT * j0 : KT * (j0 + 1)],
                                ident[:])
            nc.tensor.transpose(tr_ps[:, 1, 0:H], vt[:, KT * j1 : KT * (j1 + 1)],
                                ident[:])
            nc.vector.tensor_copy(vsb[:, j0, 0:H], tr_ps[:, 0, 0:H])
            nc.vector.tensor_copy(vsb[:, j1, 0:H], tr_ps[:, 1, 0:H])
        # ---- attention for q-group g ----
        o_ps = ps_o.tile([H + 1, GQ], F32)
        jmax = 4 * g + 3
        for ja in range(0, jmax + 1, 2):
            jb = ja + 1
            s_ps = ps_s.tile([128, 2, GQ], F32, tag="s")
            pt_t = pt_pool.tile([128, 2, GQ], BF16)
            qlos = []
            for half, j in enumerate((ja, jb)):
                s = j - 4 * g                   # diagonal sub-block index
                qlos.append(max(0, 128 * s))    # first valid q column
                nc.tensor.matmul(
                    s_ps[:, half, qlos[half] : GQ],
                    kt[:, KT * j : KT * (j + 1)],
                    qt[:, GQ * g + qlos[half] : GQ * (g + 1)],
                    start=True, stop=True)
            if jb - 4 * g >= 0:
                # diagonal pair: exact per-half exp (no never-written PSUM
                # columns read), then causal masks
                for half, j in enumerate((ja, jb)):
                    qlo = qlos[half]
                    nc.scalar.activation(pt_t[:, half, qlo:GQ],
                                         s_ps[:, half, qlo:GQ], EXP)
                    if j - 4 * g >= 0:
                        nc.vector.tensor_mul(pt_t[:, half, qlo : qlo + 128],
                                             pt_t[:, half, qlo : qlo + 128],
                                             mask[:])
            else:
                # full pair: one merged exp over both halves
                nc.scalar.activation(pt_t[:, :, :], s_ps[:, :, :], EXP)
            for half, j in enumerate((ja, jb)):
                qlo = qlos[half]
                nc.tensor.matmul(o_ps[:, qlo:GQ], vsb[:, j, :],
                                 pt_t[:, half, qlo:GQ],
                                 start=(j == 0), stop=(j == jmax))
        # ---- store unnormalized O^T + denominator row ----
        osb = out_pool.tile([H + 1, GQ], BF16)
        nc.vector.tensor_copy(osb[:], o_ps[:])
        nc.sync.dma_start(out=outT[:, sl], in_=osb[:])


def build():
    nc = bacc.Bacc("TRN2", target_bir_lowering=False, debug=False)
    with tile.TileContext(nc) as tc:
        with ExitStack() as ctx:
            _emit(ctx, tc)
    nc.compile()
    return nc


_NC_CACHE = None


def _get_module():
    global _NC_CACHE
    if _NC_CACHE is None:
        _NC_CACHE = build()
    return _NC_CACHE


def prep_in_maps(x, Wq, Wk, Wv):
    x = np.asarray(x, dtype=np.float32)
    Wq = np.asarray(Wq, dtype=np.float32)
    Wk = np.asarray(Wk, dtype=np.float32)
    Wv = np.asarray(Wv, dtype=np.float32)
    s = 1.0 / np.sqrt(H)
    # [C, M] -> [p, ci, M] with C = ci*128 + p
    wqk = np.ascontiguousarray(
        np.concatenate([Wq * s, Wk], axis=1).reshape(CC, 128, 2 * H)
        .transpose(1, 0, 2)).astype(BF_NP)
    wv = np.ascontiguousarray(
        Wv.reshape(CC, 128, H).transpose(1, 0, 2)).astype(BF_NP)
    maps = []
    for b in range(B):
        # xh[p, g, ci, t] = x[b][g*GQ + t, ci*128 + p]
        xh = np.ascontiguousarray(
            x[b].T.reshape(CC, 128, NG, GQ).transpose(1, 2, 0, 3)).astype(BF_NP)
        maps.append({"xh": xh, "wqk": wqk, "wv": wv})
    return maps


def assemble_out(results):
    out = np.empty((B, T, H), dtype=np.float32)
    for b in range(B):
        o = np.asarray(results[b]["outT"], dtype=np.float32)
        out[b] = (o[0:H, :] / o[H : H + 1, :]).T
    return out


def run(x, Wq, Wk, Wv, trace=False):
    nc = _get_module()
    in_maps = prep_in_maps(x, Wq, Wk, Wv)
    res = run_bass_kernel_spmd(nc, in_maps, core_ids=list(range(N_CORES)),
                               trace=trace)
    return assemble_out(res.results), res


def kernel(x, Wq, Wk, Wv):
    out, _ = run(x, Wq, Wk, Wv)
    return out


# revision 13
# speedup vs baseline: 1.0043x; 1.0043x over previous
"""Single-head causal attention (B=8, T=2048, C=1024, H=64) on 8 TRN2 NeuronCores.

Sharding: data-parallel over batch -- core b computes batch element b. No
collectives. Per core, for x_b [T, C]:
    q = x_b @ Wq / sqrt(H); k = x_b @ Wk; v = x_b @ Wv
    out = softmax(causal(q @ k.T)) @ v

v4 design (v2 ~66.6 us, fp32r baseline ~102 us). On this toolchain the PE
serializes LDWEIGHTS with the preceding matmul (single weight buffer;
walrus' ldw-opt pass is disabled), so array tile-packing does NOT overlap
matmuls -- the v2 matmul stream is already near the serial floor. v4 keeps
v2's matmul structure and attacks startup latency, engine-queue pressure
and the tail instead:
  - All matmul operands bf16; PSUM accumulation fp32.
  - Host pre-shuffles x into xh[p, g, ci, t]. Per-group SBUF tiles; group
    0 arrives as two half-DMAs so the first projection starts ~6 us
    earlier. Weights + constants ride a different DMA ring (scalar's)
    than the bulk x transfers (sync's).
  - ACT does exp only; k-tile pairs share one [128, 2, 512] PSUM tile and
    full (non-diagonal) pairs get a single merged exp call, halving ACT
    instruction+semaphore pressure.
  - All PSUM->SBUF casts/copies/masks are explicit DVE ops.
  - Softmax denominator rides row 64 of the O matmul (ones column in V).
    Output is UNNORMALIZED [65, T] bf16 (64 rows O^T, row 64 = denom);
    the division happens on the host. No reciprocal / broadcast on device.
"""

from contextlib import ExitStack

import numpy as np
import ml_dtypes

import concourse.mybir as mybir
import concourse.tile as tile
from concourse import bacc
from concourse.bass_utils import run_bass_kernel_spmd
from concourse.masks import make_identity, make_upper_triangular

B, T, C, H = 8, 2048, 1024, 64
N_CORES = 8
GQ = 512          # q-group width (PSUM bank)
NG = T // GQ      # 4 q-groups
KT = 128          # k-tile size
CC = C // 128     # 8 contraction chunks
F32 = mybir.dt.float32
BF16 = mybir.dt.bfloat16
EXP = mybir.ActivationFunctionType.Exp
BF_NP = ml_dtypes.bfloat16


def _emit(ctx, tc):
    nc = tc.nc
    xh = nc.dram_tensor("xh", [128, NG, CC, GQ], BF16, kind="ExternalInput").ap()
    wqk = nc.dram_tensor("wqk", [128, CC, 2 * H], BF16, kind="ExternalInput").ap()
    wv = nc.dram_tensor("wv", [128, CC, H], BF16, kind="ExternalInput").ap()
    # rows 0..63: unnormalized O^T; row 64: softmax denominator
    outT = nc.dram_tensor("outT", [H + 1, T], BF16, kind="ExternalOutput").ap()

    const = ctx.enter_context(tc.tile_pool(name="const", bufs=1))
    persist = ctx.enter_context(tc.tile_pool(name="persist", bufs=1))
    pt_pool = ctx.enter_context(tc.tile_pool(name="pt", bufs=4))
    out_pool = ctx.enter_context(tc.tile_pool(name="outp", bufs=2))
    # PSUM (8 banks): qk 2 + v 1 + s-pairs 2x2 (transposes share) + o 1
    ps_qk = ctx.enter_context(tc.tile_pool(name="ps_qk", bufs=2, space="PSUM"))
    ps_v = ctx.enter_context(tc.tile_pool(name="ps_v", bufs=1, space="PSUM"))
    ps_s = ctx.enter_context(tc.tile_pool(name="ps_s", bufs=2, space="PSUM"))
    ps_o = ctx.enter_context(tc.tile_pool(name="ps_o", bufs=1, space="PSUM"))

    # weights first on the sync ring (small; the scalar ring signals
    # completion several us late -- measured 5 us for 0.26 MB)
    wqk_sb = const.tile([128, CC, 2 * H], BF16)
    nc.sync.dma_start(out=wqk_sb[:], in_=wqk)
    wv_sb = const.tile([128, CC, H], BF16)
    nc.sync.dma_start(out=wv_sb[:], in_=wv)
    # mask[p, f] = 1.0 where p <= f else 0 : keep k_local <= q_local.
    mask_f = const.tile([128, 128], F32)
    make_upper_triangular(nc, mask_f[:], val=1.0, diag=True)
    mask = const.tile([128, 128], BF16)
    nc.vector.tensor_copy(mask[:], mask_f[:])
    ident_f = const.tile([H, H], F32)
    make_identity(nc, ident_f[:])
    ident = const.tile([H, H], BF16)
    nc.vector.tensor_copy(ident[:], ident_f[:])

    # per-group x tiles; group 0 in two half-DMAs so ci=0..3 land early
    xsb = []
    for g in range(NG):
        xg = persist.tile([128, CC, GQ], BF16, tag=f"x{g}")
        if g == 0:
            nc.sync.dma_start(out=xg[:, 0 : CC // 2], in_=xh[:, g, 0 : CC // 2])
            nc.sync.dma_start(out=xg[:, CC // 2 : CC], in_=xh[:, g, CC // 2 : CC])
        else:
            nc.sync.dma_start(out=xg[:], in_=xh[:, g])
        xsb.append(xg)

    # PE warmup: dummy matmuls on the mask tile while the first x DMA is in
    # flight -- ramps the PE clock gate (cold 1.2 GHz -> warm 2.4 GHz takes
    # ~3.4 us of sustained activity) so the real projections start warm.
    # Outputs go to an s-pool slot that is not read until much later.
    warm_ps = ps_s.tile([128, 2, GQ], F32, tag="s")
    for wi in range(14):
        nc.tensor.matmul(warm_ps[:, wi % 2, 0:128], mask[:], mask[:],
                         start=True, stop=True)

    qt = persist.tile([H, T], BF16)             # Q^T (pre-scaled by 1/sqrt(H))
    kt = persist.tile([H, T], BF16)             # K^T
    vt = persist.tile([H, T], BF16)             # V^T
    vsb = persist.tile([128, T // KT, H + 1], BF16)  # V natural tiles + ones col
    nc.vector.memset(vsb[:, :, H : H + 1], 1.0)

    for g in range(NG):
        sl = slice(GQ * g, GQ * (g + 1))
        # ---- projections for t-span sl ----
        qk_ps = ps_qk.tile([128, GQ], F32)
        v_ps = ps_v.tile([H, GQ], F32)
        for ci in range(CC):
            nc.tensor.matmul(qk_ps[:], wqk_sb[:, ci, :], xsb[g][:, ci],
                             start=(ci == 0), stop=(ci == CC - 1))
            nc.tensor.matmul(v_ps[:], wv_sb[:, ci, :], xsb[g][:, ci],
                             start=(ci == 0), stop=(ci == CC - 1))
        nc.vector.tensor_copy(qt[:, sl], qk_ps[0:H, :])
        nc.vector.tensor_copy(kt[:, sl], qk_ps[H:128, :])
        nc.vector.tensor_copy(vt[:, sl], v_ps[:, :])
        # ---- V^T -> natural V tiles (PE transpose, pairs share an s-slot) ----
        for jj in range(0, 4, 2):
            j0, j1 = 4 * g + jj, 4 * g + jj + 1
            # [128, 2, 1024] bf16 = 2KB per half: each transpose's start=True
            # zeroes a whole PSUM bank, so each half gets its own bank
            tr_ps = ps_s.tile([128, 2, 1024], BF16, tag="s")
            nc.tensor.transpose(tr_ps[:, 0, 0:H], vt[:, KT * j0 : KT * (j0 + 1)],
                                ident[:])
            nc.tensor.transpose(tr_ps[:, 1, 0:H], vt[:, KT * j1 : KT * (j1 + 1)],
                                ident[:])
            nc.vector.tensor_copy(vsb[:, j0, 0:H], tr_ps[:, 0, 0:H])
            nc.vector.tensor_copy(vsb[:, j1, 0:H], tr_ps[:, 1, 0:H])
        # ---- attention for q-group g ----
        o_ps = ps_o.tile([H + 1, GQ], F32)
        jmax = 4 * g + 3
        for ja in range(0, jmax + 1, 2):
            jb = ja + 1
            s_ps = ps_s.tile([128, 2, GQ], F32, tag="s")
            pt_t = pt_pool.tile([128, 2, GQ], BF16)
            qlos = []
            for half, j in enumerate((ja, jb)):
                s = j - 4 * g                   # diagonal sub-block index
                qlos.append(max(0, 128 * s))    # first valid q column
                nc.tensor.matmul(
                    s_ps[:, half, qlos[half] : GQ],
                    kt[:, KT * j : KT * (j + 1)],
                    qt[:, GQ * g + qlos[half] : GQ * (g + 1)],
                    start=True, stop=True)
            if jb - 4 * g >= 0:
                # diagonal pair: exact per-half exp (no never-written PSUM
                # columns read), then causal masks
                for half, j in enumerate((ja, jb)):
                    qlo = qlos[half]
                    nc.scalar.activation(pt_t[:, half, qlo:GQ],
                                         s_ps[:, half, qlo:GQ], EXP)
                    if j - 4 * g >= 0:
                        nc.vector.tensor_mul(pt_t[:, half, qlo : qlo + 128],
                                             pt_t[:, half, qlo : qlo + 128],
                                             mask[:])
            else:
                # full pair: one merged exp over both halves
                nc.scalar.activation(pt_t[:, :, :], s_ps[:, :, :], EXP)
            for half, j in enumerate((ja, jb)):
                qlo = qlos[half]
                nc.tensor.matmul(o_ps[:, qlo:GQ], vsb[:, j, :],
                                 pt_t[:, half, qlo:GQ],
                                 start=(j == 0), stop=(j == jmax))
        # ---- store unnormalized O^T + denominator row ----
        osb = out_pool.tile([H + 1, GQ], BF16)
        nc.vector.tensor_copy(osb[:], o_ps[:])
        nc.sync.dma_start(out=outT[:, sl], in_=osb[:])


def build():
    nc = bacc.Bacc("TRN2", target_bir_lowering=False, debug=False)
    with tile.TileContext(nc) as tc:
        with ExitStack() as ctx:
            _emit(ctx, tc)
    nc.compile()
    return nc


_NC_CACHE = None


def _get_module():
    global _NC_CACHE
    if _NC_CACHE is None:
        _NC_CACHE = build()
    return _NC_CACHE


def prep_in_maps(x, Wq, Wk, Wv):
    x = np.asarray(x, dtype=np.float32)
    Wq = np.asarray(Wq, dtype=np.float32)
    Wk = np.asarray(Wk, dtype=np.float32)
    Wv = np.asarray(Wv, dtype=np.float32)
    s = 1.0 / np.sqrt(H)
    # [C, M] -> [p, ci, M] with C = ci*128 + p
    wqk = np.ascontiguousarray(
        np.concatenate([Wq * s, Wk], axis=1).reshape(CC, 128, 2 * H)
        .transpose(1, 0, 2)).astype(BF_NP)
    wv = np.ascontiguousarray(
        Wv.reshape(CC, 128, H).transpose(1, 0, 2)).astype(BF_NP)
    maps = []
    for b in range(B):
        # xh[p, g, ci, t] = x[b][g*GQ + t, ci*128 + p]
        xh = np.ascontiguousarray(
            x[b].T.reshape(CC, 128, NG, GQ).transpose(1, 2, 0, 3)).astype(BF_NP)
        maps.append({"xh": xh, "wqk": wqk, "wv": wv})
    return maps


def assemble_out(results):
    out = np.empty((B, T, H), dtype=np.float32)
    for b in range(B):
        o = np.asarray(results[b]["outT"], dtype=np.float32)
        out[b] = (o[0:H, :] / o[H : H + 1, :]).T
    return out


def run(x, Wq, Wk, Wv, trace=False):
    nc = _get_module()
    in_maps = prep_in_maps(x, Wq, Wk, Wv)
    res = run_bass_kernel_spmd(nc, in_maps, core_ids=list(range(N_CORES)),
                               trace=trace)
    return assemble_out(res.results), res


def kernel(x, Wq, Wk, Wv):
    out, _ = run(x, Wq, Wk, Wv)
    return out


# revision 18
# speedup vs baseline: 1.1516x; 1.1467x over previous
"""Single-head causal attention (B=8, T=2048, C=1024, H=64) on 8 TRN2 NeuronCores.

Sharding: data-parallel over batch -- core b computes batch element b. No
collectives. Per core, for x_b [T, C]:
    q = x_b @ Wq / sqrt(H); k = x_b @ Wk; v = x_b @ Wv
    out = softmax(causal(q @ k.T)) @ v

v4 design (v2 ~66.6 us, fp32r baseline ~102 us). On this toolchain the PE
serializes LDWEIGHTS with the preceding matmul (single weight buffer;
walrus' ldw-opt pass is disabled), so array tile-packing does NOT overlap
matmuls -- the v2 matmul stream is already near the serial floor. v4 keeps
v2's matmul structure and attacks startup latency, engine-queue pressure
and the tail instead:
  - All matmul operands bf16; PSUM accumulation fp32.
  - Host pre-shuffles x into xh[p, g, ci, t]. Per-group SBUF tiles; group
    0 arrives as two half-DMAs so the first projection starts ~6 us
    earlier. Weights + constants ride a different DMA ring (scalar's)
    than the bulk x transfers (sync's).
  - ACT does exp only; k-tile pairs share one [128, 2, 512] PSUM tile and
    full (non-diagonal) pairs get a single merged exp call, halving ACT
    instruction+semaphore pressure.
  - All PSUM->SBUF casts/copies/masks are explicit DVE ops.
  - Softmax denominator rides row 64 of the O matmul (ones column in V).
    Output is UNNORMALIZED [65, T] bf16 (64 rows O^T, row 64 = denom);
    the division happens on the host. No reciprocal / broadcast on device.
"""

from contextlib import ExitStack

import numpy as np
import ml_dtypes

import concourse.mybir as mybir
import concourse.tile as tile
from concourse import bacc
from concourse.bass_utils import run_bass_kernel_spmd
from concourse.masks import make_identity, make_upper_triangular

B, T, C, H = 8, 2048, 1024, 64
N_CORES = 8
GQ = 512          # q-group width (PSUM bank)
NG = T // GQ      # 4 q-groups
KT = 128          # k-tile size
CC = C // 128     # 8 contraction chunks
F32 = mybir.dt.float32
BF16 = mybir.dt.bfloat16
EXP = mybir.ActivationFunctionType.Exp
BF_NP = ml_dtypes.bfloat16


def _emit(ctx, tc):
    nc = tc.nc
    xh = nc.dram_tensor("xh", [128, NG, CC, GQ], BF16, kind="ExternalInput").ap()
    # [Wq*s | Wk | Wv] packed: one DMA descriptor (~0.8us of serial sync-queue
    # time each) instead of three
    wall = nc.dram_tensor("wall", [128, CC, 3 * H], BF16, kind="ExternalInput").ap()
    # rows 0..63: unnormalized O^T; row 64: softmax denominator
    outT = nc.dram_tensor("outT", [H + 1, T], BF16, kind="ExternalOutput").ap()

    const = ctx.enter_context(tc.tile_pool(name="const", bufs=1))
    persist = ctx.enter_context(tc.tile_pool(name="persist", bufs=1))
    pt_pool = ctx.enter_context(tc.tile_pool(name="pt", bufs=4))
    out_pool = ctx.enter_context(tc.tile_pool(name="outp", bufs=2))
    # PSUM (8 banks): qk 2 + v 1 + s-pairs 2x2 (transposes share) + o 1
    ps_qk = ctx.enter_context(tc.tile_pool(name="ps_qk", bufs=2, space="PSUM"))
    ps_v = ctx.enter_context(tc.tile_pool(name="ps_v", bufs=1, space="PSUM"))
    ps_s = ctx.enter_context(tc.tile_pool(name="ps_s", bufs=2, space="PSUM"))
    ps_o = ctx.enter_context(tc.tile_pool(name="ps_o", bufs=1, space="PSUM"))

    # weights first on the sync ring (small; the scalar ring signals
    # completion several us late -- measured 5 us for 0.26 MB)
    wall_sb = const.tile([128, CC, 3 * H], BF16)
    nc.sync.dma_start(out=wall_sb[:], in_=wall)
    # mask[p, f] = 1.0 where p <= f else 0 : keep k_local <= q_local.
    mask_f = const.tile([128, 128], F32)
    make_upper_triangular(nc, mask_f[:], val=1.0, diag=True)
    mask = const.tile([128, 128], BF16)
    nc.vector.tensor_copy(mask[:], mask_f[:])
    ident_f = const.tile([H, H], F32)
    make_identity(nc, ident_f[:])
    ident = const.tile([H, H], BF16)
    nc.vector.tensor_copy(ident[:], ident_f[:])

    # per-group x tiles; group 0 in two half-DMAs so ci=0..3 land early
    xsb = []
    for g in range(NG):
        xg = persist.tile([128, CC, GQ], BF16, tag=f"x{g}")
        if g == 0:
            nc.sync.dma_start(out=xg[:, 0 : CC // 2], in_=xh[:, g, 0 : CC // 2])
            nc.sync.dma_start(out=xg[:, CC // 2 : CC], in_=xh[:, g, CC // 2 : CC])
        else:
            nc.sync.dma_start(out=xg[:], in_=xh[:, g])
        xsb.append(xg)

    qt = persist.tile([H, T], BF16)             # Q^T (pre-scaled by 1/sqrt(H))
    kt = persist.tile([H, T], BF16)             # K^T
    vt = persist.tile([H, T], BF16)             # V^T
    vsb = persist.tile([128, T // KT, H + 1], BF16)  # V natural tiles + ones col
    nc.vector.memset(vsb[:, :, H : H + 1], 1.0)

    for g in range(NG):
        sl = slice(GQ * g, GQ * (g + 1))
        # ---- projections for t-span sl ----
        qk_ps = ps_qk.tile([128, GQ], F32)
        v_ps = ps_v.tile([H, GQ], F32)
        for ci in range(CC):
            nc.tensor.matmul(qk_ps[:], wall_sb[:, ci, 0 : 2 * H], xsb[g][:, ci],
                             start=(ci == 0), stop=(ci == CC - 1))
            nc.tensor.matmul(v_ps[:], wall_sb[:, ci, 2 * H : 3 * H], xsb[g][:, ci],
                             start=(ci == 0), stop=(ci == CC - 1))
        nc.vector.tensor_copy(qt[:, sl], qk_ps[0:H, :])
        nc.vector.tensor_copy(kt[:, sl], qk_ps[H:128, :])
        nc.vector.tensor_copy(vt[:, sl], v_ps[:, :])
        # ---- V^T -> natural V tiles (PE transpose, pairs share an s-slot) ----
        for jj in range(0, 4, 2):
            j0, j1 = 4 * g + jj, 4 * g + jj + 1
            # [128, 2, 1024] bf16 = 2KB per half: each transpose's start=True
            # zeroes a whole PSUM bank, so each half gets its own bank
            tr_ps = ps_s.tile([128, 2, 1024], BF16, tag="s")
            nc.tensor.transpose(tr_ps[:, 0, 0:H], vt[:, KT * j0 : KT * (j0 + 1)],
                                ident[:])
            nc.tensor.transpose(tr_ps[:, 1, 0:H], vt[:, KT * j1 : KT * (j1 + 1)],
                                ident[:])
            nc.vector.tensor_copy(vsb[:, j0, 0:H], tr_ps[:, 0, 0:H])
            nc.vector.tensor_copy(vsb[:, j1, 0:H], tr_ps[:, 1, 0:H])
        # ---- attention for q-group g ----
        o_ps = ps_o.tile([H + 1, GQ], F32)
        jmax = 4 * g + 3
        for ja in range(0, jmax + 1, 2):
            jb = ja + 1
            s_ps = ps_s.tile([128, 2, GQ], F32, tag="s")
            pt_t = pt_pool.tile([128, 2, GQ], BF16)
            qlos = []
            for half, j in enumerate((ja, jb)):
                s = j - 4 * g                   # diagonal sub-block index
                qlos.append(max(0, 128 * s))    # first valid q column
                nc.tensor.matmul(
                    s_ps[:, half, qlos[half] : GQ],
                    kt[:, KT * j : KT * (j + 1)],
                    qt[:, GQ * g + qlos[half] : GQ * (g + 1)],
                    start=True, stop=True)
            if jb - 4 * g >= 0:
                # diagonal pair: exact per-half exp (no never-written PSUM
                # columns read), then causal masks
                for half, j in enumerate((ja, jb)):
                    qlo = qlos[half]
                    nc.scalar.activation(pt_t[:, half, qlo:GQ],
                                         s_ps[:, half, qlo:GQ], EXP)
                    if j - 4 * g >= 0:
                        nc.vector.tensor_mul(pt_t[:, half, qlo : qlo + 128],
                                             pt_t[:, half, qlo : qlo + 128],
                                             mask[:])
            else:
                # full pair: one merged exp over both halves
                nc.scalar.activation(pt_t[:, :, :], s_ps[:, :, :], EXP)
            for half, j in enumerate((ja, jb)):
                qlo = qlos[half]
                nc.tensor.matmul(o_ps[:, qlo:GQ], vsb[:, j, :],
                                 pt_t[:, half, qlo:GQ],
                                 start=(j == 0), stop=(j == jmax))
        # ---- store unnormalized O^T + denominator row ----
        osb = out_pool.tile([H + 1, GQ], BF16)
        nc.vector.tensor_copy(osb[:], o_ps[:])
        nc.sync.dma_start(out=outT[:, sl], in_=osb[:])


def build():
    nc = bacc.Bacc("TRN2", target_bir_lowering=False, debug=False)
    with tile.TileContext(nc) as tc:
        with ExitStack() as ctx:
            _emit(ctx, tc)
    nc.compile()
    return nc


_NC_CACHE = None


def _get_module():
    global _NC_CACHE
    if _NC_CACHE is None:
        _NC_CACHE = build()
    return _NC_CACHE


def prep_in_maps(x, Wq, Wk, Wv):
    x = np.asarray(x, dtype=np.float32)
    Wq = np.asarray(Wq, dtype=np.float32)
    Wk = np.asarray(Wk, dtype=np.float32)
    Wv = np.asarray(Wv, dtype=np.float32)
    s = 1.0 / np.sqrt(H)
    # [C, M] -> [p, ci, M] with C = ci*128 + p
    wall = np.ascontiguousarray(
        np.concatenate([Wq * s, Wk, Wv], axis=1).reshape(CC, 128, 3 * H)
        .transpose(1, 0, 2)).astype(BF_NP)
    maps = []
    for b in range(B):
        # xh[p, g, ci, t] = x[b][g*GQ + t, ci*128 + p]
        xh = np.ascontiguousarray(
            x[b].T.reshape(CC, 128, NG, GQ).transpose(1, 2, 0, 3)).astype(BF_NP)
        maps.append({"xh": xh, "wall": wall})
    return maps


def assemble_out(results):
    out = np.empty((B, T, H), dtype=np.float32)
    for b in range(B):
        o = np.asarray(results[b]["outT"], dtype=np.float32)
        out[b] = (o[0:H, :] / o[H : H + 1, :]).T
    return out


def run(x, Wq, Wk, Wv, trace=False):
    nc = _get_module()
    in_maps = prep_in_maps(x, Wq, Wk, Wv)
    res = run_bass_kernel_spmd(nc, in_maps, core_ids=list(range(N_CORES)),
                               trace=trace)
    return assemble_out(res.results), res


def kernel(x, Wq, Wk, Wv):
    out, _ = run(x, Wq, Wk, Wv)
    return out
